# revision 3
# baseline (speedup 1.0000x reference)
"""BinaryLinear (straight-through sign(w)) kernel for Trainium2, 8 NeuronCores.

Computes out = x @ sign(w).T + b for
  x: [8192, 2048] f32, w: [4096, 2048] f32, b: [4096] f32 -> out [8192, 4096] f32.

Sharding: 4-way data parallel (batch) x 2-way tensor parallel (out_features).
Each core computes a [2048, 2048] block of the output:
  out[bi*2048:(bi+1)*2048, fi*2048:(fi+1)*2048]
    = x_shard @ sign(w_shard).T + b_shard.

Per-core device kernel — mixed-precision contraction, fp32 accumulate:
  - K is split K8 + K16.  The first K8 in_features use fp8e4 (e4m3) operands
    with perf_mode=DoubleRow (2 fp8 weights per PE cell, 256-deep virtual
    contraction): 2x the PE FLOP rate of fp16.  sign(w) in {-1,0,1} is exact
    in e4m3; only x pays quantization error (~2.65% rms per element, measured
    on the fixed problem data), diluted to 2.65%*sqrt(K8/2048) of the output.
    K8=1024 -> 1.87% rel l2, inside the 2e-2 budget.  The remaining K16
    columns run in fp16 (exact to ~3e-4).
  - the whole sign(w)^T shard ships and lives in SBUF as ONE fp8 plane
    (1 byte per weight): the DoubleRow phase reads it as fp8 pairs, and the
    fp16 phase uses it directly as the MOVING operand against fp16 x (the
    PE upconverts per-operand; cost and numerics verified on hardware).
    It loads once over the ACT HWDGE ring, n0 first in consumption-order
    chunks, so the startup block streams in lockstep with the x loads;
  - x^T tiles stream through multi-buffered pools on the SP HWDGE ring,
    with the next m-tile prefetched two n-blocks early;
  - PSUM accumulation is m-subtile-major: each [128, 512] output tile runs
    its whole K reduction back-to-back, so evictions stagger and the tail
    after the last matmul is one eviction + one store instead of four;
  - bias is added during the PSUM->SBUF copyback on the vector engine.
"""

from contextlib import ExitStack

import numpy as np

# Full problem shapes (hardcoded per the grading contract).
M, K, N = 8192, 2048, 4096
P_BATCH, P_FEAT = 4, 2  # 4 x 2 core grid
MC, NC = M // P_BATCH, N // P_FEAT  # 2048, 2048 per-core block
N_CORES = P_BATCH * P_FEAT
P = 128
K8 = 1024  # in_features contracted in fp8e4 DoubleRow (multiple of 512)
K16 = K - K8  # in_features contracted in fp16
N_WARM = 5  # HAM warmup matmuls
TAIL_SPLIT = True  # halve the final eviction+store chain
WARM_MEMSET = True  # zero the warmup scratch (off: garbage inputs, discarded)
TAIL_GP = False  # second final half-eviction on gpsimd (parallel with DVE)
W_ORDER = 6  # ACT queue: n0 fp8 chunks, n0 fp16-phase in 2 chunks, bias, n1+ halves
PSUM_BUFS = 6  # PSUM rotation depth (8 banks total)
EV_BUFS = 4  # eviction sbuf tile rotation depth
PREFETCH_NB = 2  # n-block at which the next m-tile's x loads are issued
SPLIT16 = True  # halve m0's fp16 x loads for finer startup unblocking
TAIL_LAST = 256  # columns in the final store of the last tile (256 = verified config)


def build_nc(mc: int = MC, nc_dim: int = NC, reps: int = 1):
    """Build + compile the per-core Bass module:
    out[mc, nc_dim] = x8^T.T @ w8 + x16^T.T @ w16 + bias.

    reps > 1 repeats the whole computation (for slope-based benchmarking)."""
    import concourse.mybir as mybir
    import concourse.tile as tile
    from concourse import bacc

    TB = 512  # m/n tile width of the pre-blocked host layouts
    KT = 512  # k-tile width
    KS = KT // P  # k-subtiles per k-tile (4)
    ko8, ko16 = K8 // P, K16 // P
    k8_tiles, k16_tiles = K8 // KT, K16 // KT
    m_tiles = mc // TB
    m_subs = TB // P  # m-subtiles per m-tile (4)
    n_blocks = nc_dim // TB

    nc = bacc.Bacc("TRN2", target_bir_lowering=False, debug=False)
    DR = mybir.MatmulPerfMode.DoubleRow

    # x inputs arrive pre-blocked on the host (see _pack_blocks): each
    # [P, KS, TB] block is fully contiguous in DRAM, so every DMA has
    # large per-partition descriptor runs instead of strided ones.
    xt8 = nc.dram_tensor(
        "xt8", [m_tiles, k8_tiles, P, KS, TB], mybir.dt.float8e4,
        kind="ExternalInput",
    )
    xt16 = nc.dram_tensor(
        "xt16", [m_tiles, k16_tiles, P, KS, TB], mybir.dt.float16,
        kind="ExternalInput",
    )
    # w uses an n-block-major layout ([nb, p, ko, n]) so each n-block's
    # preload is contiguous per partition on BOTH sides.  The whole sign
    # matrix ships as ONE fp8 tensor: the DoubleRow phase needs fp8 anyway,
    # and the fp16 phase uses the fp8 plane directly as its MOVING operand
    # (sign values are exact in e4m3; the PE upconverts per-operand, and the
    # matmul cost is set by the moving dtype at the same 1 row/cycle as
    # fp16) — 1 byte/weight instead of 2 through the startup DMA crunch.
    ko = ko8 + ko16
    wt = nc.dram_tensor(
        "wt", [n_blocks, P, ko, TB], mybir.dt.float8e4, kind="ExternalInput"
    )
    bias = nc.dram_tensor("bias", [nc_dim], mybir.dt.float32, kind="ExternalInput")
    out = nc.dram_tensor("out", [mc, nc_dim], mybir.dt.float32, kind="ExternalOutput")

    with tile.TileContext(nc) as tc, ExitStack() as ctx:
        # HAM warmup: the PE clock is gated (0.65/1.2 GHz) until ~3 us of
        # sustained activity. Start throwaway matmuls as early as possible
        # (gpsimd memset is ~100 ns) so the ramp burns while the first
        # operand DMAs are in flight. The scratch SBUF pool stays OPEN so
        # its slot is never reused; only the PSUM bank is returned.
        warm_sb = ctx.enter_context(tc.tile_pool(name="warm_sb", bufs=1))
        scratch = warm_sb.tile([P, 512], mybir.dt.float16)
        if WARM_MEMSET:
            nc.vector.memset(scratch[:], 0.0)
        with tc.tile_pool(name="warm_ps", bufs=1, space="PSUM") as wps_pool:
            ps = wps_pool.tile([P, 512], mybir.dt.float32)
            for _ in range(N_WARM):
                nc.tensor.matmul(
                    ps[:], scratch[:, :P], scratch[:], start=True, stop=True
                )

        const = ctx.enter_context(tc.tile_pool(name="const", bufs=1))
        x8_pool = ctx.enter_context(tc.tile_pool(name="x8", bufs=2 * k8_tiles))
        x16_pool = ctx.enter_context(tc.tile_pool(name="x16", bufs=2 * k16_tiles))
        ev_pool = ctx.enter_context(tc.tile_pool(name="ev", bufs=EV_BUFS))
        psum = ctx.enter_context(tc.tile_pool(name="psum", bufs=PSUM_BUFS, space="PSUM"))

        # Whole sign(w)^T shard resident in SBUF, n-block-major, as an fp8
        # plane (first K8 in_features) and an fp16 plane (rest):
        #   w8_sb[p, nb, o, j]  = sign(w)^T[o*128 + p, nb*TB + j],  o <  ko8
        #   w16_sb[p, nb, o, j] = sign(w)^T[K8 + o*128 + p, nb*TB + j]
        w_sb = const.tile([P, n_blocks, ko, TB], mybir.dt.float8e4)

        bias_sb = const.tile([P, nc_dim], mybir.dt.float32)

        # The cost model serializes ALL DMA transfers on one shared pipe and
        # alternates HWDGE descriptor generation between the SP and ACT
        # queues; in-queue gens are FIFO, so transfer order tracks issue
        # order, and the early transfer order is the startup critical path.
        # SWDGE is avoided for w entirely — its independent transfers would
        # jump ahead of the critical n0 stream.  W_ORDER=6 (n0 fp8 k-tile
        # chunks, n0 fp16-phase in TWO chunks — fewer early descriptor-gen
        # slots advance the n1 transfer past its consumer — then bias, then
        # n1..n3 halves) is verified by CoreSim (exact output, race-clean)
        # and bit-exact repeated HW runs.
        def w_chunk(nb, lo, hi):
            nc.scalar.dma_start(out=w_sb[:, nb, lo:hi], in_=wt.ap()[nb, :, lo:hi])

        def bias_row():
            nc.scalar.dma_start(out=bias_sb[:1, :], in_=bias.ap()[None, :])

        if W_ORDER == 5:  # first fp8 chunk halved: first matmul unblocks earlier
            w_chunk(0, 0, KS // 2)
            w_chunk(0, KS // 2, KS)
            for kt in range(1, k8_tiles):
                w_chunk(0, kt * KS, (kt + 1) * KS)
        else:
            for kt in range(k8_tiles):
                w_chunk(0, kt * KS, (kt + 1) * KS)
        if W_ORDER in (0, 2):
            bias_row()
        if W_ORDER == 6:  # n0 fp16-phase in 2 chunks: frees 2 early gen slots
            for kt in range(k16_tiles):
                w_chunk(0, ko8 + kt * KS, ko8 + (kt + 1) * KS)
        else:
            for h in range(2 * k16_tiles):
                w_chunk(0, ko8 + h * (KS // 2), ko8 + (h + 1) * (KS // 2))
        if W_ORDER not in (0, 2):
            bias_row()
        if W_ORDER == 5:
            w_chunk(1, 0, ko8 // 2)
            w_chunk(1, ko8 // 2, ko8)
            w_chunk(1, ko8, ko)
        elif W_ORDER in (2, 3):  # n1 quarters
            for q in range(4):
                w_chunk(1, q * (ko // 4), (q + 1) * (ko // 4))
        elif W_ORDER == 4:  # n1 fp8 quarters + fp16 half
            w_chunk(1, 0, ko8 // 2)
            w_chunk(1, ko8 // 2, ko8)
            w_chunk(1, ko8, ko)
        else:  # n1 halves
            w_chunk(1, 0, ko8)
            w_chunk(1, ko8, ko)
        for nb in range(2, n_blocks):
            w_chunk(nb, 0, ko8)
            w_chunk(nb, ko8, ko)

        nc.gpsimd.partition_broadcast(bias_sb[:], bias_sb[:1, :])

        out_t = out.ap().rearrange("(o p) n -> p o n", p=P)

        def load_m(m, split16=False):
            """Issue the x-tile loads for m-tile m on the SP HWDGE ring.
            split16 halves the fp16 loads so the startup block's first fp16
            matmuls unblock a k-subtile-pair earlier."""
            t8 = []
            for kt in range(k8_tiles):
                t = x8_pool.tile([P, KS, TB], mybir.dt.float8e4, tag="x8")
                nc.sync.dma_start(out=t[:], in_=xt8.ap()[m, kt])
                t8.append(t)
            t16 = []
            for kt in range(k16_tiles):
                t = x16_pool.tile([P, KS, TB], mybir.dt.float16, tag="x16")
                if split16 and (split16 != 2 or kt == 0):
                    h = KS // 2
                    nc.sync.dma_start(out=t[:, :h], in_=xt16.ap()[m, kt, :, :h])
                    nc.sync.dma_start(out=t[:, h:], in_=xt16.ap()[m, kt, :, h:])
                else:
                    nc.sync.dma_start(out=t[:], in_=xt16.ap()[m, kt])
                t16.append(t)
            return t8, t16

        def mm_fp8(pt, x8_t, nb, kt, kk, sub, start):
            ms = slice(sub * P, (sub + 1) * P)
            nc.tensor.matmul(
                pt[:],
                x8_t[kt][:, 2 * kk : 2 * kk + 2, ms],
                w_sb[:, nb, kt * KS + 2 * kk : kt * KS + 2 * kk + 2, :],
                start=start,
                stop=False,
                perf_mode=DR,
            )

        def mm_fp16(pt, x16_t, nb, kt, s, sub, stop):
            ms = slice(sub * P, (sub + 1) * P)
            nc.tensor.matmul(
                pt[:],
                x16_t[kt][:, s, ms],
                w_sb[:, nb, ko8 + kt * KS + s, :],
                start=False,
                stop=stop,
            )

        def evict(pt, m, nb, sub):
            po = m * m_subs + sub
            last_block = m == m_tiles - 1 and nb == n_blocks - 1
            ev = ev_pool.tile([P, TB], mybir.dt.float32, tag="ev")
            if TAIL_SPLIT and last_block and sub == m_subs - 1:
                # Final tile: asymmetric split of the add+store chain — the
                # LAST store (whose completion latency ends the kernel) is
                # made tiny so its whole gen+transfer+completion chain starts
                # and ends earliest; store gens go to different HWDGE rings.
                cut = TB - TAIL_LAST
                for (lo, hi), eng in (((0, cut), nc.sync), ((cut, TB), nc.scalar)):
                    cs = slice(lo, hi)
                    nc.vector.tensor_add(
                        out=ev[:, cs],
                        in0=pt[:, cs],
                        in1=bias_sb[:, nb * TB + lo : nb * TB + hi],
                    )
                    eng.dma_start(
                        out=out_t[:, po : po + 1, nb * TB + lo : nb * TB + hi],
                        in_=ev[:, None, cs],
                    )
                return
            nc.vector.tensor_add(
                out=ev[:],
                in0=pt[:],
                in1=bias_sb[:, nb * TB : (nb + 1) * TB],
            )
            # Alternate the last block's store gens across the two HWDGE
            # rings so the final chain never queues behind a prior gen.
            eng = nc.scalar if (last_block and sub % 2 == 1) else nc.sync
            eng.dma_start(
                out=out_t[:, po : po + 1, nb * TB : (nb + 1) * TB],
                in_=ev[:, None, :],
            )

        for _ in range(reps):
            nxt = load_m(0, split16=SPLIT16)
            for m in range(m_tiles):
                x8_t, x16_t = nxt
                for nb in range(n_blocks):
                    if nb == PREFETCH_NB and m + 1 < m_tiles:
                        # Prefetch the next m-tile two n-blocks early: the
                        # loads jump the SP ring ahead of this m-tile's
                        # remaining stores (which have slack).
                        nxt = load_m(m + 1)
                    if m == 0 and nb == 0:
                        # Startup block runs k-major so every arriving k-tile
                        # chunk unlocks 4 subtiles of PE work (the operand
                        # stream is the critical path here).
                        pts = [
                            psum.tile([P, TB], mybir.dt.float32, name=f"pts_{i}", tag="ps")
                            for i in range(m_subs)
                        ]
                        for kt in range(k8_tiles):
                            for kk in range(KS // 2):
                                for sub in range(m_subs):
                                    mm_fp8(pts[sub], x8_t, nb, kt, kk, sub,
                                           start=(kt == 0 and kk == 0))
                        for kt in range(k16_tiles):
                            for s in range(KS):
                                for sub in range(m_subs):
                                    mm_fp16(pts[sub], x16_t, nb, kt, s, sub,
                                            stop=(kt == k16_tiles - 1 and s == KS - 1))
                        for sub in range(m_subs):
                            evict(pts[sub], m, nb, sub)
                        continue
                    # Steady state runs m-subtile-major: each [128, 512]
                    # output tile does its whole K reduction back-to-back, so
                    # evictions stagger (and the tail after the last matmul is
                    # one eviction + one store instead of four).
                    for sub in range(m_subs):
                        pt = psum.tile([P, TB], mybir.dt.float32, tag="ps")
                        for kt in range(k8_tiles):
                            for kk in range(KS // 2):
                                mm_fp8(pt, x8_t, nb, kt, kk, sub,
                                       start=(kt == 0 and kk == 0))
                        for kt in range(k16_tiles):
                            for s in range(KS):
                                mm_fp16(pt, x16_t, nb, kt, s, sub,
                                        stop=(kt == k16_tiles - 1 and s == KS - 1))
                        evict(pt, m, nb, sub)

    nc.compile()
    return nc


def _pack_w_nblocks(a: np.ndarray, tb: int = 512) -> np.ndarray:
    """[N, K] row-major -> [N//tb, 128, K//128, tb] with
    block[nb, p, o, j] = a[nb*tb + j, o*128 + p]; per-partition-contiguous
    [ko, tb] planes -> large DMA descriptor runs."""
    n, k = a.shape
    v = a.reshape(n // tb, tb, k // P, P)
    return np.ascontiguousarray(v.transpose(0, 3, 2, 1))


def _pack_blocks(a: np.ndarray, tb: int = 512) -> np.ndarray:
    """[F, K] row-major -> [F//tb, K//ktw, 128, ks, tb] DMA-contiguous blocks.

    block[ft, kt, p, s, j] = a[ft*tb + j, kt*ktw + s*128 + p], i.e. each
    [128, ks, tb] block is one fully-contiguous DMA source with K on the
    partition dim (a^T layout within the block)."""
    f, k = a.shape
    ktw = min(512, k)
    kts, ks = k // ktw, ktw // P
    v = a.reshape(f // tb, tb, kts, ks, P)
    return np.ascontiguousarray(v.transpose(0, 2, 4, 3, 1))


_NC_CACHE = None


def _get_nc():
    global _NC_CACHE
    if _NC_CACHE is None:
        _NC_CACHE = build_nc()
    return _NC_CACHE


def make_in_maps(x: np.ndarray, w: np.ndarray, b: np.ndarray) -> list:
    import ml_dtypes

    x = np.asarray(x, dtype=np.float32)
    w = np.asarray(w, dtype=np.float32)
    b = np.asarray(b, dtype=np.float32)

    f8 = ml_dtypes.float8_e4m3
    f16 = np.float16
    s = np.sign(w)

    # Unique DMA-blocked shards (x per batch group, sign(w) per feature
    # group), packed in parallel (numpy releases the GIL on these copies).
    from concurrent.futures import ThreadPoolExecutor

    def pack_x8(bi):
        return _pack_blocks(x[bi * MC : (bi + 1) * MC, :K8].astype(f8))

    def pack_x16(bi):
        return _pack_blocks(x[bi * MC : (bi + 1) * MC, K8:].astype(f16))

    def pack_w(fi):
        # [n_blocks, P, ko, TB] fp8 of the whole sign shard.
        return _pack_w_nblocks(s[fi * NC : (fi + 1) * NC].astype(f8))

    with ThreadPoolExecutor(max_workers=8) as pool:
        x8_f = [pool.submit(pack_x8, bi) for bi in range(P_BATCH)]
        x16_f = [pool.submit(pack_x16, bi) for bi in range(P_BATCH)]
        w_f = [pool.submit(pack_w, fi) for fi in range(P_FEAT)]
        x8_shards = [f.result() for f in x8_f]
        x16_shards = [f.result() for f in x16_f]
        w_shards = [f.result() for f in w_f]
    b_shards = [np.ascontiguousarray(b[fi * NC : (fi + 1) * NC]) for fi in range(P_FEAT)]

    in_maps = []
    for c in range(N_CORES):
        bi, fi = divmod(c, P_FEAT)
        in_maps.append(
            {
                "xt8": x8_shards[bi],
                "xt16": x16_shards[bi],
                "wt": w_shards[fi],
                "bias": b_shards[fi],
            }
        )
    return in_maps


def kernel(x: np.ndarray, w: np.ndarray, b: np.ndarray) -> np.ndarray:
    from concourse.bass_utils import run_bass_kernel_spmd

    in_maps = make_in_maps(x, w, b)
    nc = _get_nc()
    try:
        results = run_bass_kernel_spmd(
            nc, in_maps, core_ids=list(range(N_CORES))
        ).results
    except Exception:
        # One retry for transient runtime/relay failures.
        results = run_bass_kernel_spmd(
            nc, in_maps, core_ids=list(range(N_CORES))
        ).results

    out = np.empty((M, N), dtype=np.float32)
    for c in range(N_CORES):
        bi, fi = divmod(c, P_FEAT)
        out[bi * MC : (bi + 1) * MC, fi * NC : (fi + 1) * NC] = results[c]["out"]
    return out



# revision 4
# speedup vs baseline: 1.5489x; 1.5489x over previous
"""BinaryLinear (straight-through sign(w)) kernel for Trainium2, 8 NeuronCores.

Computes out = x @ sign(w).T + b for
  x: [8192, 2048] f32, w: [4096, 2048] f32, b: [4096] f32 -> out [8192, 4096] f32.

Sharding: 4-way data parallel (batch) x 2-way tensor parallel (out_features).
Each core computes a [2048, 2048] block of the output:
  out[bi*2048:(bi+1)*2048, fi*2048:(fi+1)*2048]
    = x_shard @ sign(w_shard).T + b_shard.

Per-core device kernel — two-term fp8 contraction, fp32 accumulate:
  - EVERYTHING runs as fp8e4 (e4m3) DoubleRow matmuls (2 fp8 weights per PE
    cell, 256-deep virtual contraction, 0.5 cycles/row): 4x the per-K-column
    PE rate of fp16.  sign(w) in {-1,0,1} is exact in e4m3.
  - term 1: x8 = e4m3(x) over the FULL K=2048 (8 DR matmuls per output tile);
  - term 2: r8 = e4m3(x - e4m3(x)) over the first KR=1024 in_features
    (4 DR matmuls), which cancels the fp8 quantization error on those
    columns to ~0.07% per element.  The uncorrected 1024 columns contribute
    2.65% * sqrt(1024/2048) = 1.87% output rel l2 — same error structure as
    the old K8/K16 mixed-precision split at 3072 instead of 5120 PE cycles
    per [128, 512] output tile.
  - the whole sign(w)^T shard ships and lives in SBUF as ONE fp8 plane
    (1 byte per weight), n-block-major; term 2 reuses the SAME w rows
    (ko slices 0..7) as term 1's first half — no extra w traffic.
    It loads once over the ACT HWDGE ring, n0 first in k-tile chunks so the
    startup block streams in lockstep with the x loads;
  - x8/r8 tiles stream through multi-buffered pools on the SP HWDGE ring,
    with the next m-tile prefetched two n-blocks early;
  - PSUM accumulation is m-subtile-major: each [128, 512] output tile runs
    its whole K reduction back-to-back, so evictions stagger and the tail
    after the last matmul is one eviction + one store instead of four;
  - bias is added during the PSUM->SBUF copyback on the vector engine, which
    also narrows to fp16: the output ships as fp16 (rounding adds ~1e-4 rel,
    output magnitudes ~260 max vs fp16 range 65504) and the host upcasts,
    halving the store traffic.
"""

from contextlib import ExitStack

import numpy as np

# Full problem shapes (hardcoded per the grading contract).
M, K, N = 8192, 2048, 4096
P_BATCH, P_FEAT = 4, 2  # 4 x 2 core grid
MC, NC = M // P_BATCH, N // P_FEAT  # 2048, 2048 per-core block
N_CORES = P_BATCH * P_FEAT
P = 128
KR = 1024  # residual-corrected in_features (multiple of 512)
N_WARM = 5  # HAM warmup matmuls
TAIL_SPLIT = True  # halve the final eviction+store chain
WARM_MEMSET = True  # zero the warmup scratch (off: garbage inputs, discarded)
PSUM_BUFS = 6  # PSUM rotation depth (8 banks total)
EV_BUFS = 4  # eviction sbuf tile rotation depth
PREFETCH_NB = 2  # n-block at which the next m-tile's x loads are issued
SPLIT_X0 = True  # halve m0's first x8 load for finer startup unblocking
TAIL_LAST = 256  # columns in the final store of the last tile


def build_nc(mc: int = MC, nc_dim: int = NC, reps: int = 1):
    """Build + compile the per-core Bass module:
    out[mc, nc_dim] = fp16(x8^T.T @ w + r8^T.T @ w[:KR] + bias).

    reps > 1 repeats the whole computation (for slope-based benchmarking)."""
    import concourse.mybir as mybir
    import concourse.tile as tile
    from concourse import bacc

    TB = 512  # m/n tile width of the pre-blocked host layouts
    KT = 512  # k-tile width
    KS = KT // P  # k-subtiles per k-tile (4)
    ko = K // P  # 16
    k_tiles = K // KT  # 4
    kr_tiles = KR // KT  # 2
    m_tiles = mc // TB
    m_subs = TB // P  # m-subtiles per m-tile (4)
    n_blocks = nc_dim // TB

    nc = bacc.Bacc("TRN2", target_bir_lowering=False, debug=False)
    DR = mybir.MatmulPerfMode.DoubleRow

    # x inputs arrive pre-blocked on the host (see _pack_blocks): each
    # [P, KS, TB] block is fully contiguous in DRAM, so every DMA has
    # large per-partition descriptor runs instead of strided ones.
    xt8 = nc.dram_tensor(
        "xt8", [m_tiles, k_tiles, P, KS, TB], mybir.dt.float8e4,
        kind="ExternalInput",
    )
    rt8 = nc.dram_tensor(
        "rt8", [m_tiles, kr_tiles, P, KS, TB], mybir.dt.float8e4,
        kind="ExternalInput",
    )
    # w uses an n-block-major layout ([nb, p, ko, n]) so each n-block's
    # preload is contiguous per partition on BOTH sides.  The whole sign
    # matrix ships as ONE fp8 tensor; the residual term reuses ko slices
    # 0..kor-1.
    wt = nc.dram_tensor(
        "wt", [n_blocks, P, ko, TB], mybir.dt.float8e4, kind="ExternalInput"
    )
    bias = nc.dram_tensor("bias", [nc_dim], mybir.dt.float32, kind="ExternalInput")
    out = nc.dram_tensor("out", [mc, nc_dim], mybir.dt.float16, kind="ExternalOutput")

    with tile.TileContext(nc) as tc, ExitStack() as ctx:
        # HAM warmup: the PE clock is gated (0.65/1.2 GHz) until ~3 us of
        # sustained activity. Start throwaway matmuls as early as possible
        # (gpsimd memset is ~100 ns) so the ramp burns while the first
        # operand DMAs are in flight. The scratch SBUF pool stays OPEN so
        # its slot is never reused; only the PSUM bank is returned.
        warm_sb = ctx.enter_context(tc.tile_pool(name="warm_sb", bufs=1))
        scratch = warm_sb.tile([P, 512], mybir.dt.float16)
        if WARM_MEMSET:
            nc.vector.memset(scratch[:], 0.0)
        with tc.tile_pool(name="warm_ps", bufs=1, space="PSUM") as wps_pool:
            ps = wps_pool.tile([P, 512], mybir.dt.float32)
            for _ in range(N_WARM):
                nc.tensor.matmul(
                    ps[:], scratch[:, :P], scratch[:], start=True, stop=True
                )

        const = ctx.enter_context(tc.tile_pool(name="const", bufs=1))
        x8_pool = ctx.enter_context(tc.tile_pool(name="x8", bufs=2 * k_tiles))
        r8_pool = ctx.enter_context(tc.tile_pool(name="r8", bufs=2 * kr_tiles))
        ev_pool = ctx.enter_context(tc.tile_pool(name="ev", bufs=EV_BUFS))
        psum = ctx.enter_context(tc.tile_pool(name="psum", bufs=PSUM_BUFS, space="PSUM"))

        # Whole sign(w)^T shard resident in SBUF, n-block-major, one fp8
        # plane: w_sb[p, nb, o, j] = sign(w)^T[o*128 + p, nb*TB + j].
        w_sb = const.tile([P, n_blocks, ko, TB], mybir.dt.float8e4)

        bias_sb = const.tile([P, nc_dim], mybir.dt.float32)

        # The cost model serializes ALL DMA transfers on one shared pipe and
        # alternates HWDGE descriptor generation between the SP and ACT
        # queues; in-queue gens are FIFO, so transfer order tracks issue
        # order, and the early transfer order is the startup critical path.
        # ACT ring: n0's w in k-tile chunks (consumption order for the
        # k-major startup block), bias, then n1..n3 in halves.
        def w_chunk(nb, lo, hi):
            nc.scalar.dma_start(out=w_sb[:, nb, lo:hi], in_=wt.ap()[nb, :, lo:hi])

        def bias_row():
            nc.scalar.dma_start(out=bias_sb[:1, :], in_=bias.ap()[None, :])

        for kt in range(k_tiles):
            w_chunk(0, kt * KS, (kt + 1) * KS)
        bias_row()
        for nb in range(1, n_blocks):
            w_chunk(nb, 0, ko // 2)
            w_chunk(nb, ko // 2, ko)

        nc.gpsimd.partition_broadcast(bias_sb[:], bias_sb[:1, :])

        out_t = out.ap().rearrange("(o p) n -> p o n", p=P)

        def load_m(m, split0=False):
            """Issue the x-tile loads for m-tile m on the SP HWDGE ring.
            split0 halves the first fp8 load so the startup block's first
            matmuls unblock a k-subtile-pair earlier."""
            t8 = []
            for kt in range(k_tiles):
                t = x8_pool.tile([P, KS, TB], mybir.dt.float8e4, tag="x8")
                if split0 and kt == 0:
                    h = KS // 2
                    nc.sync.dma_start(out=t[:, :h], in_=xt8.ap()[m, kt, :, :h])
                    nc.sync.dma_start(out=t[:, h:], in_=xt8.ap()[m, kt, :, h:])
                else:
                    nc.sync.dma_start(out=t[:], in_=xt8.ap()[m, kt])
                t8.append(t)
            tr = []
            for kt in range(kr_tiles):
                t = r8_pool.tile([P, KS, TB], mybir.dt.float8e4, tag="r8")
                nc.sync.dma_start(out=t[:], in_=rt8.ap()[m, kt])
                tr.append(t)
            return t8, tr

        def mm_dr(pt, x_t, nb, kt, kk, sub, start, stop=False):
            ms = slice(sub * P, (sub + 1) * P)
            nc.tensor.matmul(
                pt[:],
                x_t[kt][:, 2 * kk : 2 * kk + 2, ms],
                w_sb[:, nb, kt * KS + 2 * kk : kt * KS + 2 * kk + 2, :],
                start=start,
                stop=stop,
                perf_mode=DR,
            )

        def evict(pt, m, nb, sub):
            po = m * m_subs + sub
            last_block = m == m_tiles - 1 and nb == n_blocks - 1
            ev = ev_pool.tile([P, TB], mybir.dt.float16, tag="ev")
            if TAIL_SPLIT and last_block and sub == m_subs - 1:
                # Final tile: asymmetric split of the add+store chain — the
                # LAST store (whose completion latency ends the kernel) is
                # made tiny so its whole gen+transfer+completion chain starts
                # and ends earliest; store gens go to different HWDGE rings.
                cut = TB - TAIL_LAST
                for (lo, hi), eng in (((0, cut), nc.sync), ((cut, TB), nc.scalar)):
                    cs = slice(lo, hi)
                    nc.vector.tensor_add(
                        out=ev[:, cs],
                        in0=pt[:, cs],
                        in1=bias_sb[:, nb * TB + lo : nb * TB + hi],
                    )
                    eng.dma_start(
                        out=out_t[:, po : po + 1, nb * TB + lo : nb * TB + hi],
                        in_=ev[:, None, cs],
                    )
                return
            nc.vector.tensor_add(
                out=ev[:],
                in0=pt[:],
                in1=bias_sb[:, nb * TB : (nb + 1) * TB],
            )
            # Alternate the last block's store gens across the two HWDGE
            # rings so the final chain never queues behind a prior gen.
            eng = nc.scalar if (last_block and sub % 2 == 1) else nc.sync
            eng.dma_start(
                out=out_t[:, po : po + 1, nb * TB : (nb + 1) * TB],
                in_=ev[:, None, :],
            )

        def accumulate(pt, x8_t, r8_t, nb, sub):
            for kt in range(k_tiles):
                for kk in range(KS // 2):
                    mm_dr(pt, x8_t, nb, kt, kk, sub, start=(kt == 0 and kk == 0))
            for kt in range(kr_tiles):
                for kk in range(KS // 2):
                    mm_dr(pt, r8_t, nb, kt, kk, sub, start=False,
                          stop=(kt == kr_tiles - 1 and kk == KS // 2 - 1))

        for _ in range(reps):
            nxt = load_m(0, split0=SPLIT_X0)
            for m in range(m_tiles):
                x8_t, r8_t = nxt
                for nb in range(n_blocks):
                    if nb == PREFETCH_NB and m + 1 < m_tiles:
                        # Prefetch the next m-tile two n-blocks early: the
                        # loads jump the SP ring ahead of this m-tile's
                        # remaining stores (which have slack).
                        nxt = load_m(m + 1)
                    if m == 0 and nb == 0:
                        # Startup block runs k-major so every arriving k-tile
                        # chunk unlocks 4 subtiles of PE work (the operand
                        # stream is the critical path here).
                        pts = [
                            psum.tile([P, TB], mybir.dt.float32, name=f"pts_{i}", tag="ps")
                            for i in range(m_subs)
                        ]
                        for kt in range(k_tiles):
                            for kk in range(KS // 2):
                                for sub in range(m_subs):
                                    mm_dr(pts[sub], x8_t, nb, kt, kk, sub,
                                          start=(kt == 0 and kk == 0))
                        for kt in range(kr_tiles):
                            for kk in range(KS // 2):
                                for sub in range(m_subs):
                                    mm_dr(pts[sub], r8_t, nb, kt, kk, sub,
                                          start=False,
                                          stop=(kt == kr_tiles - 1 and kk == KS // 2 - 1))
                        for sub in range(m_subs):
                            evict(pts[sub], m, nb, sub)
                        continue
                    # Steady state runs m-subtile-major: each [128, 512]
                    # output tile does its whole K reduction back-to-back, so
                    # evictions stagger (and the tail after the last matmul is
                    # one eviction + one store instead of four).
                    for sub in range(m_subs):
                        pt = psum.tile([P, TB], mybir.dt.float32, tag="ps")
                        accumulate(pt, x8_t, r8_t, nb, sub)
                        evict(pt, m, nb, sub)

    nc.compile()
    return nc


def _pack_w_nblocks(a: np.ndarray, tb: int = 512) -> np.ndarray:
    """[N, K] row-major -> [N//tb, 128, K//128, tb] with
    block[nb, p, o, j] = a[nb*tb + j, o*128 + p]; per-partition-contiguous
    [ko, tb] planes -> large DMA descriptor runs."""
    n, k = a.shape
    v = a.reshape(n // tb, tb, k // P, P)
    return np.ascontiguousarray(v.transpose(0, 3, 2, 1))


def _pack_blocks(a: np.ndarray, tb: int = 512) -> np.ndarray:
    """[F, K] row-major -> [F//tb, K//ktw, 128, ks, tb] DMA-contiguous blocks.

    block[ft, kt, p, s, j] = a[ft*tb + j, kt*ktw + s*128 + p], i.e. each
    [128, ks, tb] block is one fully-contiguous DMA source with K on the
    partition dim (a^T layout within the block)."""
    f, k = a.shape
    ktw = min(512, k)
    kts, ks = k // ktw, ktw // P
    v = a.reshape(f // tb, tb, kts, ks, P)
    return np.ascontiguousarray(v.transpose(0, 2, 4, 3, 1))


_NC_CACHE = None


def _get_nc():
    global _NC_CACHE
    if _NC_CACHE is None:
        _NC_CACHE = build_nc()
    return _NC_CACHE


def make_in_maps(x: np.ndarray, w: np.ndarray, b: np.ndarray) -> list:
    import ml_dtypes

    x = np.asarray(x, dtype=np.float32)
    w = np.asarray(w, dtype=np.float32)
    b = np.asarray(b, dtype=np.float32)

    f8 = ml_dtypes.float8_e4m3
    s = np.sign(w)

    # Unique DMA-blocked shards (x per batch group, sign(w) per feature
    # group), packed in parallel (numpy releases the GIL on these copies).
    from concurrent.futures import ThreadPoolExecutor

    def pack_x8(bi):
        xs = x[bi * MC : (bi + 1) * MC]
        x8 = xs.astype(f8)
        r8 = (xs[:, :KR] - x8[:, :KR].astype(np.float32)).astype(f8)
        return _pack_blocks(x8), _pack_blocks(r8)

    def pack_w(fi):
        # [n_blocks, P, ko, TB] fp8 of the whole sign shard.
        return _pack_w_nblocks(s[fi * NC : (fi + 1) * NC].astype(f8))

    with ThreadPoolExecutor(max_workers=8) as pool:
        x_f = [pool.submit(pack_x8, bi) for bi in range(P_BATCH)]
        w_f = [pool.submit(pack_w, fi) for fi in range(P_FEAT)]
        x_shards = [f.result() for f in x_f]
        w_shards = [f.result() for f in w_f]
    b_shards = [np.ascontiguousarray(b[fi * NC : (fi + 1) * NC]) for fi in range(P_FEAT)]

    in_maps = []
    for c in range(N_CORES):
        bi, fi = divmod(c, P_FEAT)
        in_maps.append(
            {
                "xt8": x_shards[bi][0],
                "rt8": x_shards[bi][1],
                "wt": w_shards[fi],
                "bias": b_shards[fi],
            }
        )
    return in_maps


def kernel(x: np.ndarray, w: np.ndarray, b: np.ndarray) -> np.ndarray:
    from concourse.bass_utils import run_bass_kernel_spmd

    in_maps = make_in_maps(x, w, b)
    nc = _get_nc()
    try:
        results = run_bass_kernel_spmd(
            nc, in_maps, core_ids=list(range(N_CORES))
        ).results
    except Exception:
        # One retry for transient runtime/relay failures.
        results = run_bass_kernel_spmd(
            nc, in_maps, core_ids=list(range(N_CORES))
        ).results

    out = np.empty((M, N), dtype=np.float32)
    for c in range(N_CORES):
        bi, fi = divmod(c, P_FEAT)
        out[bi * MC : (bi + 1) * MC, fi * NC : (fi + 1) * NC] = results[c][
            "out"
        ].astype(np.float32)
    return out


# revision 37
# speedup vs baseline: 1.6093x; 1.0390x over previous
"""BinaryLinear (straight-through sign(w)) kernel for Trainium2, 8 NeuronCores.

Computes out = x @ sign(w).T + b for
  x: [8192, 2048] f32, w: [4096, 2048] f32, b: [4096] f32 -> out [8192, 4096] f32.

Sharding: 4-way data parallel (batch) x 2-way tensor parallel (out_features).
Each core computes a [2048, 2048] block of the output:
  out[bi*2048:(bi+1)*2048, fi*2048:(fi+1)*2048]
    = x_shard @ sign(w_shard).T + b_shard.

Per-core device kernel — two-term fp8 contraction, fp32 accumulate:
  - EVERYTHING runs as fp8e4 (e4m3) DoubleRow matmuls (2 fp8 weights per PE
    cell, 256-deep virtual contraction, 0.5 cycles/row): 4x the per-K-column
    PE rate of fp16.  sign(w) in {-1,0,1} is exact in e4m3.
  - term 1: x8 = e4m3(x) over the FULL K=2048 (8 DR matmuls per output tile);
  - term 2: r8 = e4m3(x - e4m3(x)) over the first KR=1024 in_features
    (4 DR matmuls), which cancels the fp8 quantization error on those
    columns to ~0.07% per element.  The uncorrected 1024 columns contribute
    2.65% * sqrt(1024/2048) = 1.87% output rel l2 — same error structure as
    the old K8/K16 mixed-precision split at 3072 instead of 5120 PE cycles
    per [128, 512] output tile.
  - the whole sign(w)^T shard ships and lives in SBUF as ONE fp8 plane
    (1 byte per weight), n-block-major; term 2 reuses the SAME w rows
    (ko slices 0..7) as term 1's first half — no extra w traffic.
    It loads once over the ACT HWDGE ring, n0 first in k-tile chunks so the
    startup block streams in lockstep with the x loads;
  - x8/r8 tiles stream through multi-buffered pools on the SP HWDGE ring,
    with the next m-tile prefetched two n-blocks early;
  - PSUM accumulation is m-subtile-major: each [128, 512] output tile runs
    its whole K reduction back-to-back, so evictions stagger and the tail
    after the last matmul is one eviction + one store instead of four;
  - bias is added during the PSUM->SBUF copyback on the vector engine, which
    also narrows to fp16: the output ships as fp16 (rounding adds ~1e-4 rel,
    output magnitudes ~260 max vs fp16 range 65504) and the host upcasts,
    halving the store traffic.
"""

from contextlib import ExitStack

import numpy as np

# Full problem shapes (hardcoded per the grading contract).
M, K, N = 8192, 2048, 4096
P_BATCH, P_FEAT = 4, 2  # 4 x 2 core grid
MC, NC = M // P_BATCH, N // P_FEAT  # 2048, 2048 per-core block
N_CORES = P_BATCH * P_FEAT
P = 128
KR = 1024  # residual-corrected in_features (multiple of 512)
N_WARM = 5  # HAM warmup matmuls
WARM_DR = True  # warmup matmuls in fp8 DoubleRow (half the PE occupancy)
SB_POOL_EVICT = False  # gpsimd cannot read PSUM on hardware (sim-only idea)
TAIL_SPLIT = False  # split final store: serial adds + per-store sem cost more
TAIL_GP = False  # gpsimd add for the final half-eviction: slower than DVE (0.42 eff)
WARM_MEMSET = True  # zero the warmup scratch (off: garbage inputs, discarded)
PSUM_BUFS = 8  # PSUM rotation depth (8 banks total)
EV_BUFS = 12  # eviction sbuf tile rotation depth (stores lag the adds)
PREFETCH_NB = 3  # n-block at which the next m-tile's x loads are issued
SPLIT_X0 = True  # halve m0's first x8 load for finer startup unblocking
SPLIT_W0 = True  # halve n0's first w chunk so the first matmul unblocks early
KMAJOR_NBLOCKS = 2  # first n-blocks of m0 run k-major (consume as chunks land)
TAIL_LAST = 256  # columns in the final store of the last tile


def build_nc(mc: int = MC, nc_dim: int = NC, reps: int = 1):
    """Build + compile the per-core Bass module:
    out[mc, nc_dim] = fp16(x8^T.T @ w + r8^T.T @ w[:KR] + bias).

    reps > 1 repeats the whole computation (for slope-based benchmarking)."""
    import concourse.mybir as mybir
    import concourse.tile as tile
    from concourse import bacc

    TB = 512  # m/n tile width of the pre-blocked host layouts
    KT = 512  # k-tile width
    KS = KT // P  # k-subtiles per k-tile (4)
    ko = K // P  # 16
    k_tiles = K // KT  # 4
    kr_tiles = KR // KT  # 2
    m_tiles = mc // TB
    m_subs = TB // P  # m-subtiles per m-tile (4)
    n_blocks = nc_dim // TB

    nc = bacc.Bacc("TRN2", target_bir_lowering=False, debug=False)
    DR = mybir.MatmulPerfMode.DoubleRow

    # x inputs arrive pre-blocked on the host (see _pack_blocks): each
    # [P, KS, TB] block is fully contiguous in DRAM, so every DMA has
    # large per-partition descriptor runs instead of strided ones.
    xt8 = nc.dram_tensor(
        "xt8", [m_tiles, k_tiles, P, KS, TB], mybir.dt.float8e4,
        kind="ExternalInput",
    )
    rt8 = nc.dram_tensor(
        "rt8", [m_tiles, kr_tiles, P, KS, TB], mybir.dt.float8e4,
        kind="ExternalInput",
    )
    # w uses an n-block-major layout ([nb, p, ko, n]) so each n-block's
    # preload is contiguous per partition on BOTH sides.  The whole sign
    # matrix ships as ONE fp8 tensor; the residual term reuses ko slices
    # 0..kor-1.
    wt = nc.dram_tensor(
        "wt", [n_blocks, P, ko, TB], mybir.dt.float8e4, kind="ExternalInput"
    )
    bias = nc.dram_tensor("bias", [nc_dim], mybir.dt.float32, kind="ExternalInput")
    out = nc.dram_tensor("out", [mc, nc_dim], mybir.dt.float16, kind="ExternalOutput")

    with tile.TileContext(nc) as tc, ExitStack() as ctx:
        # HAM warmup: the PE clock is gated (0.65/1.2 GHz) until ~3 us of
        # sustained activity. Start throwaway matmuls as early as possible
        # (gpsimd memset is ~100 ns) so the ramp burns while the first
        # operand DMAs are in flight. The scratch SBUF pool stays OPEN so
        # its slot is never reused; only the PSUM bank is returned.
        warm_sb = ctx.enter_context(tc.tile_pool(name="warm_sb", bufs=1))
        if WARM_DR:
            scratch = warm_sb.tile([P, 2, 512], mybir.dt.float8e4)
            if WARM_MEMSET:
                nc.gpsimd.memset(scratch[:], 0.0)
            with tc.tile_pool(name="warm_ps", bufs=1, space="PSUM") as wps_pool:
                ps = wps_pool.tile([P, 512], mybir.dt.float32)
                for _ in range(N_WARM):
                    nc.tensor.matmul(
                        ps[:], scratch[:, :, :P], scratch[:], start=True,
                        stop=True, perf_mode=DR,
                    )
        else:
            scratch = warm_sb.tile([P, 512], mybir.dt.float16)
            if WARM_MEMSET:
                nc.gpsimd.memset(scratch[:], 0.0)
            with tc.tile_pool(name="warm_ps", bufs=1, space="PSUM") as wps_pool:
                ps = wps_pool.tile([P, 512], mybir.dt.float32)
                for _ in range(N_WARM):
                    nc.tensor.matmul(
                        ps[:], scratch[:, :P], scratch[:], start=True, stop=True
                    )

        const = ctx.enter_context(tc.tile_pool(name="const", bufs=1))
        x8_pool = ctx.enter_context(tc.tile_pool(name="x8", bufs=3 * k_tiles))
        r8_pool = ctx.enter_context(tc.tile_pool(name="r8", bufs=3 * kr_tiles))
        ev_pool = ctx.enter_context(tc.tile_pool(name="ev", bufs=EV_BUFS))
        psum = ctx.enter_context(tc.tile_pool(name="psum", bufs=PSUM_BUFS, space="PSUM"))

        # Whole sign(w)^T shard resident in SBUF, n-block-major, one fp8
        # plane: w_sb[p, nb, o, j] = sign(w)^T[o*128 + p, nb*TB + j].
        w_sb = const.tile([P, n_blocks, ko, TB], mybir.dt.float8e4)

        bias_sb = const.tile([P, nc_dim], mybir.dt.float32)

        # The cost model serializes ALL DMA transfers on one shared pipe and
        # alternates HWDGE descriptor generation between the SP and ACT
        # queues; in-queue gens are FIFO, so transfer order tracks issue
        # order, and the early transfer order is the startup critical path.
        # ACT ring: bias (tiny, needed for its gpsimd broadcast well before
        # the first eviction), n0's w in k-tile chunks and m0's r8 (both in
        # the k-major startup block's consumption order), n1's w in k-tile
        # chunks (n1 also runs k-major), then n2/n3 in halves.
        # SP ring: m0's x8 k-tiles (first one split), then prefetched
        # m-tiles. The SP/ACT alternation interleaves w and x chunk arrivals
        # in lockstep with the startup block's k-major consumption.
        def w_chunk(nb, lo, hi):
            nc.scalar.dma_start(out=w_sb[:, nb, lo:hi], in_=wt.ap()[nb, :, lo:hi])

        def bias_row():
            nc.scalar.dma_start(out=bias_sb[:1, :], in_=bias.ap()[None, :])

        # ALL loads ride the ONE ACT-in DGE queue, issued in exact global
        # consumption order — the per-queue FIFO at the descriptor generator
        # means transfer order IS issue order, and a second load queue would
        # round-robin pipe slots to tiles that aren't needed yet.  Stores
        # ride the SP ring's outbound queue, which is independent.
        def load_x8(m, kts=None):
            t8 = []
            for kt in kts or range(k_tiles):
                t = x8_pool.tile([P, KS, TB], mybir.dt.float8e4, tag="x8")
                nc.scalar.dma_start(out=t[:], in_=xt8.ap()[m, kt])
                t8.append(t)
            return t8

        def load_x8_tail(m):
            return load_x8(m, kts=range(2, k_tiles))

        def load_r8(m):
            tr = []
            for kt in range(kr_tiles):
                t = r8_pool.tile([P, KS, TB], mybir.dt.float8e4, tag="r8")
                nc.scalar.dma_start(out=t[:], in_=rt8.ap()[m, kt])
                tr.append(t)
            return tr

        # m0's startup stream: the superblock runs n-blocks
        # 0..KMAJOR_NBLOCKS-1 fused k-major over all 8 PSUM banks, so each
        # arriving x k-tile feeds KMAJOR_NBLOCKS*8 matmuls — the stream
        # (x + KMAJOR_NBLOCKS w planes per k-tile) and the PE nearly pace.
        x8_m0 = []
        if SPLIT_W0:
            w_chunk(0, 0, KS // 2)
        _x0 = x8_pool.tile([P, KS, TB], mybir.dt.float8e4, tag="x8")
        h = KS // 2
        nc.scalar.dma_start(out=_x0[:, :h], in_=xt8.ap()[0, 0, :, :h])
        if SPLIT_W0:
            w_chunk(0, KS // 2, KS)
        else:
            w_chunk(0, 0, KS)
        nc.scalar.dma_start(out=_x0[:, h:], in_=xt8.ap()[0, 0, :, h:])
        x8_m0.append(_x0)
        # bias rides after the first-matmul-critical chunks (its gen slot
        # would otherwise delay them); broadcast is done long before the
        # first eviction needs it.
        bias_row()
        nc.gpsimd.partition_broadcast(bias_sb[:], bias_sb[:1, :])
        for nb in range(1, KMAJOR_NBLOCKS):
            w_chunk(nb, 0, KS)
        r8_m0 = []

        def r8_m0_tile(kt):
            t = r8_pool.tile([P, KS, TB], mybir.dt.float8e4, tag="r8")
            nc.scalar.dma_start(out=t[:], in_=rt8.ap()[0, kt])
            r8_m0.append(t)

        for kt in range(1, k_tiles):
            t = x8_pool.tile([P, KS, TB], mybir.dt.float8e4, tag="x8")
            nc.scalar.dma_start(out=t[:], in_=xt8.ap()[0, kt])
            x8_m0.append(t)
            last = kt == k_tiles - 1
            for nb in range(KMAJOR_NBLOCKS):
                if last and SPLIT_W0 and nb == KMAJOR_NBLOCKS - 1:
                    # The stream's final chunk is a half w plane, so the
                    # last-arriving byte unlocks only 4 matmuls of work.
                    w_chunk(nb, kt * KS, kt * KS + KS // 2)
                    w_chunk(nb, kt * KS + KS // 2, (kt + 1) * KS)
                else:
                    w_chunk(nb, kt * KS, (kt + 1) * KS)
            if kt < k_tiles - 1:
                # r8 tiles slot in mid-stream (consumed between x k-tiles),
                # keeping the endgame free for the last x/w chunks.
                r8_m0_tile(kt - 1)
        # n2/n3 w chunks, then m1's tiles, ahead of ALL stores in the FIFO
        # (stores are issued later in program order and have slack; their
        # HWDGE gens wait on eviction data without blocking anything that
        # matters).  m2/m3 loads are issued from inside the compute loop so
        # they sit between the right store groups.
        w_chunk(2, 0, ko // 2)
        w_chunk(2, ko // 2, ko)
        w_chunk(3, 0, ko // 2)
        w_chunk(3, ko // 2, ko)
        m_tiles_x = [x8_m0, load_x8(1)]
        m_tiles_r = [r8_m0, load_r8(1)]

        out_t = out.ap().rearrange("(o p) n -> p o n", p=P)

        def mm_dr(pt, x_t, nb, kt, kk, sub, start, stop=False):
            ms = slice(sub * P, (sub + 1) * P)
            nc.tensor.matmul(
                pt[:],
                x_t[kt][:, 2 * kk : 2 * kk + 2, ms],
                w_sb[:, nb, kt * KS + 2 * kk : kt * KS + 2 * kk + 2, :],
                start=start,
                stop=stop,
                perf_mode=DR,
            )

        def evict(pt, m, nb, sub, add_eng=None):
            add_eng = add_eng or nc.vector
            po = m * m_subs + sub
            last_block = m == m_tiles - 1 and nb == n_blocks - 1
            ev = ev_pool.tile([P, TB], mybir.dt.float16, tag="ev")
            if TAIL_SPLIT and last_block and sub == m_subs - 1:
                # Final tile: split the add+store chain in two, adds running
                # in PARALLEL on DVE and gpsimd, store gens on different
                # HWDGE rings, so the post-last-matmul critical chain is a
                # half-width add + one small store.
                cut = TB - TAIL_LAST
                halves = (
                    ((0, cut), nc.vector, nc.sync),
                    ((cut, TB), nc.gpsimd if TAIL_GP else nc.vector, nc.scalar),
                )
                for (lo, hi), add_eng, eng in halves:
                    cs = slice(lo, hi)
                    add_eng.tensor_add(
                        out=ev[:, cs],
                        in0=pt[:, cs],
                        in1=bias_sb[:, nb * TB + lo : nb * TB + hi],
                    )
                    eng.dma_start(
                        out=out_t[:, po : po + 1, nb * TB + lo : nb * TB + hi],
                        in_=ev[:, None, cs],
                    )
                return
            add_eng.tensor_add(
                out=ev[:],
                in0=pt[:],
                in1=bias_sb[:, nb * TB : (nb + 1) * TB],
            )
            # Steady stores share the ACT FIFO (behind all loads); the last
            # block alternates onto the idle SP ring so the final chain
            # never queues behind a prior gen.
            eng = nc.sync if (last_block and sub % 2 == 0) else nc.scalar
            eng.dma_start(
                out=out_t[:, po : po + 1, nb * TB : (nb + 1) * TB],
                in_=ev[:, None, :],
            )

        def accumulate(pt, x8_t, r8_t, nb, sub):
            for kt in range(k_tiles):
                for kk in range(KS // 2):
                    mm_dr(pt, x8_t, nb, kt, kk, sub, start=(kt == 0 and kk == 0))
            for kt in range(kr_tiles):
                for kk in range(KS // 2):
                    mm_dr(pt, r8_t, nb, kt, kk, sub, start=False,
                          stop=(kt == kr_tiles - 1 and kk == KS // 2 - 1))

        for rep in range(reps):
            if rep > 0:
                m_tiles_x = [load_x8(0), load_x8(1)]
                m_tiles_r = [load_r8(0), load_r8(1)]
            for m in range(m_tiles):
                x8_t, r8_t = m_tiles_x[m], m_tiles_r[m]
                for nb in range(n_blocks):
                    if nb == 1 and m + 2 < m_tiles:
                        # Issue m+2's loads here: in the ACT FIFO they land
                        # after the previous m-tile's first store group and
                        # well before their compute.
                        m_tiles_x.append(load_x8(m + 2))
                        m_tiles_r.append(load_r8(m + 2))
                    if m == 0 and nb == 0 and rep == 0 and KMAJOR_NBLOCKS:
                        # Startup superblock: fused k-major over the first
                        # KMAJOR_NBLOCKS n-blocks, consuming each k-chunk for
                        # every n-block as it lands.
                        NBS = KMAJOR_NBLOCKS
                        pts = [
                            [
                                psum.tile([P, TB], mybir.dt.float32,
                                          name=f"pts_{b}_{i}", tag="ps")
                                for i in range(m_subs)
                            ]
                            for b in range(NBS)
                        ]
                        # Consumption tracks the stream: x kt0, kt1, then r
                        # k-tiles slotted between the later x k-tiles, with
                        # the x kt3 matmuls last (stop there).
                        steps = [("x", 0), ("x", 1)]
                        for kt in range(2, k_tiles):
                            steps.append(("r", kt - 2))
                            steps.append(("x", kt))
                        steps += [("r", kt) for kt in range(k_tiles - 2, kr_tiles)]
                        for si, (term, kt) in enumerate(steps):
                            t = x8_t if term == "x" else r8_t
                            for b in range(NBS):
                                for kk in range(KS // 2):
                                    for sub in range(m_subs):
                                        mm_dr(pts[b][sub], t, b, kt, kk, sub,
                                              start=(si == 0 and kk == 0),
                                              stop=(si == len(steps) - 1
                                                    and kk == KS // 2 - 1))
                        for b in range(NBS):
                            eng = (
                                nc.gpsimd
                                if SB_POOL_EVICT and b % 2 == 1
                                else nc.vector
                            )
                            for sub in range(m_subs):
                                evict(pts[b][sub], m, b, sub, add_eng=eng)
                        continue
                    if m == 0 and nb < KMAJOR_NBLOCKS and rep == 0:
                        continue  # covered by the superblock above
                    # Steady state runs m-subtile-major: each [128, 512]
                    # output tile does its whole K reduction back-to-back, so
                    # evictions stagger (and the tail after the last matmul is
                    # one eviction + one store instead of four).
                    for sub in range(m_subs):
                        pt = psum.tile([P, TB], mybir.dt.float32, tag="ps")
                        accumulate(pt, x8_t, r8_t, nb, sub)
                        evict(pt, m, nb, sub)

    nc.compile()
    return nc


def _pack_w_nblocks(a: np.ndarray, tb: int = 512) -> np.ndarray:
    """[N, K] row-major -> [N//tb, 128, K//128, tb] with
    block[nb, p, o, j] = a[nb*tb + j, o*128 + p]; per-partition-contiguous
    [ko, tb] planes -> large DMA descriptor runs."""
    n, k = a.shape
    v = a.reshape(n // tb, tb, k // P, P)
    return np.ascontiguousarray(v.transpose(0, 3, 2, 1))


def _pack_blocks(a: np.ndarray, tb: int = 512) -> np.ndarray:
    """[F, K] row-major -> [F//tb, K//ktw, 128, ks, tb] DMA-contiguous blocks.

    block[ft, kt, p, s, j] = a[ft*tb + j, kt*ktw + s*128 + p], i.e. each
    [128, ks, tb] block is one fully-contiguous DMA source with K on the
    partition dim (a^T layout within the block)."""
    f, k = a.shape
    ktw = min(512, k)
    kts, ks = k // ktw, ktw // P
    v = a.reshape(f // tb, tb, kts, ks, P)
    return np.ascontiguousarray(v.transpose(0, 2, 4, 3, 1))


_NC_CACHE = None


def _get_nc():
    global _NC_CACHE
    if _NC_CACHE is None:
        _NC_CACHE = build_nc()
    return _NC_CACHE


def make_in_maps(x: np.ndarray, w: np.ndarray, b: np.ndarray) -> list:
    import ml_dtypes

    x = np.asarray(x, dtype=np.float32)
    w = np.asarray(w, dtype=np.float32)
    b = np.asarray(b, dtype=np.float32)

    f8 = ml_dtypes.float8_e4m3
    s = np.sign(w)

    # Unique DMA-blocked shards (x per batch group, sign(w) per feature
    # group), packed in parallel (numpy releases the GIL on these copies).
    from concurrent.futures import ThreadPoolExecutor

    def pack_x8(bi):
        xs = x[bi * MC : (bi + 1) * MC]
        x8 = xs.astype(f8)
        r8 = (xs[:, :KR] - x8[:, :KR].astype(np.float32)).astype(f8)
        return _pack_blocks(x8), _pack_blocks(r8)

    def pack_w(fi):
        # [n_blocks, P, ko, TB] fp8 of the whole sign shard.
        return _pack_w_nblocks(s[fi * NC : (fi + 1) * NC].astype(f8))

    with ThreadPoolExecutor(max_workers=8) as pool:
        x_f = [pool.submit(pack_x8, bi) for bi in range(P_BATCH)]
        w_f = [pool.submit(pack_w, fi) for fi in range(P_FEAT)]
        x_shards = [f.result() for f in x_f]
        w_shards = [f.result() for f in w_f]
    b_shards = [np.ascontiguousarray(b[fi * NC : (fi + 1) * NC]) for fi in range(P_FEAT)]

    in_maps = []
    for c in range(N_CORES):
        bi, fi = divmod(c, P_FEAT)
        in_maps.append(
            {
                "xt8": x_shards[bi][0],
                "rt8": x_shards[bi][1],
                "wt": w_shards[fi],
                "bias": b_shards[fi],
            }
        )
    return in_maps


def kernel(x: np.ndarray, w: np.ndarray, b: np.ndarray) -> np.ndarray:
    from concourse.bass_utils import run_bass_kernel_spmd

    in_maps = make_in_maps(x, w, b)
    nc = _get_nc()
    try:
        results = run_bass_kernel_spmd(
            nc, in_maps, core_ids=list(range(N_CORES))
        ).results
    except Exception:
        # One retry for transient runtime/relay failures.
        results = run_bass_kernel_spmd(
            nc, in_maps, core_ids=list(range(N_CORES))
        ).results

    out = np.empty((M, N), dtype=np.float32)
    for c in range(N_CORES):
        bi, fi = divmod(c, P_FEAT)
        out[bi * MC : (bi + 1) * MC, fi * NC : (fi + 1) * NC] = results[c][
            "out"
        ].astype(np.float32)
    return out
